# revision 19
# baseline (speedup 1.0000x reference)
# DSTP-RNN Trainium2 kernel: 8-core pure data parallel (batch 512 -> 64/core).
#
# Restructuring summary (validated numerically, rel-l2 ~7e-4 vs fp32 ref):
#  - "Score" tensors are b-major: partitions = (g, b) with g in {0,1} a
#    channel-group split, b = 64 local batch rows; free dims = (ch, tau).
#  - Per-step attention score: DVE broadcast-add of e, ACT tanh, DVE mul by
#    replicated v, DVE pairwise tree-reduce over tau (all bf16).
#  - Softmax without max-subtraction (scores are small); channel-group fold
#    and per-partition normalizer duplication via tiny PE matmuls.
#  - LSTM gates in b-major psum [64, 4H] via 3 matmuls with stationary
#    activations; gate order host-permuted to [i,f,o | g]; biases folded via
#    an appended ones-row on the stationary operand (or a K=1 init matmul).
#  - All cross-partition movement via PE (transpose matmuls with identity,
#    fold/dup matmuls with 0/1 matrices); DVE/ACT stay lane-aligned.
import numpy as np
import ml_dtypes

import concourse.bacc as bacc
import concourse.mybir as mybir
import concourse.tile as tile
from concourse.bass_utils import run_bass_kernel_spmd

F32 = mybir.dt.float32
BF16 = mybir.dt.bfloat16
AX = mybir.AxisListType
OP = mybir.AluOpType
AF = mybir.ActivationFunctionType

N_CORES = 8
B = 64      # batch per core
T = 64      # encoder length
H = 128
TD = 24     # decoder steps (T_DEC + 6)
NF = 17     # driving series count
C2 = 129    # stage-2 channels (H + label)
COLS = np.array(list(range(14)) + list(range(15, 18)))
PAD_NEG = -20.0   # pad channel fill (tanh -> -1; excluded from softmax sums)


def _perm_cols(w):
    # torch gate order (i,f,g,o) -> (i,f,o,g): sigmoid block contiguous
    i, f, g, o = np.split(w, 4, axis=-1)
    return np.concatenate([i, f, o, g], axis=-1)


def _bf(x):
    return np.ascontiguousarray(np.asarray(x).astype(ml_dtypes.bfloat16))


def _f32(x):
    return np.ascontiguousarray(np.asarray(x).astype(np.float32))


def prep_weights(inp):
    w = {}
    w["Wi1R"] = _bf(np.concatenate([inp["Wi_w"].T, inp["Wi_b"][None, :]], 0))
    w["Wi2R"] = _bf(np.concatenate([inp["Wi2_w"].T * 0.5, inp["Wi2_b"][None, :]], 0))
    w["We1R"] = _f32(inp["We_w"].T * 0.5)
    w["We2R"] = _f32(inp["We2_w"].T * 0.5)
    w["WhR"] = _f32(inp["Wh_w"].T * 0.5)
    w["WxR"] = _f32(inp["Wx_w"].T * 0.5)
    w["Wxb"] = _f32(inp["Wx_b"][None, :])

    # ISO: sigmoid gates computed as tanh(x/2) -> pre-scale i,f,o cols by 0.5.
    # States are stored doubled (hS=2h, cS=2c), so weight blocks consuming
    # h/c/mid/din get an extra 0.5.
    ISO = np.concatenate([0.5 * np.ones(384), np.ones(128)]).astype(np.float32)
    g1x = _perm_cols(inp["Wih1"].T) * ISO
    b1 = _perm_cols((inp["bih1"] + inp["bhh1"])[None, :]) * ISO
    w["G1XA"] = _f32(g1x[0:9])
    w["b1row"] = _f32(b1)
    w["G1XB"] = _f32(g1x[9:17])
    w["G1H"] = _f32(_perm_cols(inp["Whh1"].T) * ISO * 0.5)

    g2x = _perm_cols(inp["Wih2"].T) * ISO * 0.5
    b2 = _perm_cols((inp["bih2"] + inp["bhh2"])[None, :]) * ISO
    w["G2XA"] = _f32(g2x[0:65])
    w["b2row"] = _f32(b2)
    w["G2XB"] = _f32(g2x[65:129])
    w["G2H"] = _f32(_perm_cols(inp["Whh2"].T) * ISO * 0.5)

    w["GdX"] = _f32(_perm_cols(inp["Wihd"].T) * ISO * 0.5)
    w["GdH"] = _f32(_perm_cols(inp["Whhd"].T) * ISO * 0.5)
    w["bdrow"] = _f32(_perm_cols((inp["bihd"] + inp["bhhd"])[None, :]) * ISO)

    w["vdup1"] = _bf(np.broadcast_to(inp["Vd_w"][0][None, :], (128, T)))
    w["vdup2"] = _bf(np.broadcast_to(inp["Vd2_w"][0][None, :], (128, T)))
    w["vdup3"] = _bf(np.broadcast_to(inp["V_w"][0][None, :], (128, H)))
    w["regw"] = _f32(inp["reg_w"][0][:, None] * 0.5)

    eye = np.eye(64, dtype=np.float32)
    w["I64dup"] = _f32(np.concatenate([eye, eye], 0))
    foldL = np.zeros((128, 64), np.float32)
    for p in range(128):
        foldL[p, p % 64] = 1.0
    w["foldLbf"] = _bf(foldL)
    foldDup = (np.arange(128)[:, None] % 64 == np.arange(128)[None, :] % 64)
    w["foldDup"] = _f32(foldDup.astype(np.float32))
    return w


def prep_core_inputs(inp, core):
    b0, b1 = core * B, (core + 1) * B
    x = np.asarray(inp["input_p_q"])[b0:b1, :T, :][:, :, COLS]   # [64,64,17]
    lab = np.asarray(inp["label_p"])[b0:b1, :T]                  # [64,64]
    d = {}
    inpT = np.ones((65, NF * B), np.float32)
    inpT[:64] = x.transpose(1, 2, 0).reshape(64, NF * B)         # [t, (c,b)]
    d["inpT"] = _bf(inpT)
    ct = x.transpose(2, 1, 0).reshape(NF, T * B)                 # [c, (t,b)]
    d["inpCTA"] = _bf(ct[0:9])
    d["inpCTB"] = _bf(ct[9:17])
    d["labelT"] = _f32(lab.T * 2.0)                                    # [t, b]
    return d


DRAM_SPECS = {
    "inpT": ([65, NF * B], BF16), "inpCTA": ([9, T * B], BF16),
    "inpCTB": ([8, T * B], BF16), "labelT": ([T, B], F32),
    "Wi1R": ([65, 64], BF16), "Wi2R": ([65, 64], BF16),
    "We1R": ([256, 64], F32), "We2R": ([256, 64], F32),
    "WhR": ([256, 128], F32), "WxR": ([128, 128], F32), "Wxb": ([1, 128], F32),
    "G1XA": ([9, 512], F32), "b1row": ([1, 512], F32), "G1XB": ([8, 512], F32), "G1H": ([128, 512], F32),
    "G2XA": ([65, 512], F32), "b2row": ([1, 512], F32), "G2XB": ([64, 512], F32), "G2H": ([128, 512], F32),
    "GdX": ([128, 512], F32), "GdH": ([128, 512], F32), "bdrow": ([1, 512], F32),
    "vdup1": ([128, T], BF16), "vdup2": ([128, T], BF16), "vdup3": ([128, H], BF16),
    "regw": ([128, 1], F32), "I64dup": ([128, 64], F32),
    "foldLbf": ([128, 64], BF16), "foldDup": ([128, 128], F32),
}


def build_nc(num_devices=N_CORES, skip_score=False, skip_tail=False, only_stages=(1, 2, 3)):
    nc = bacc.Bacc("TRN2", target_bir_lowering=False, debug=False,
                   num_devices=num_devices)
    dr = {}
    for name, (shape, dt) in DRAM_SPECS.items():
        dr[name] = nc.dram_tensor(name, shape, dt, kind="ExternalInput").ap()
    out_d = nc.dram_tensor("out", [B, 18], F32, kind="ExternalOutput").ap()

    with tile.TileContext(nc) as tc:
        # ---------- persistent SBUF ----------
        wpool = tc.alloc_tile_pool(name="wpool", bufs=1)
        sb = {}
        for name, (shape, dt) in DRAM_SPECS.items():
            if shape[0] > 128:
                assert shape[0] == 256
                for half, suf in ((0, "a"), (1, "b")):
                    key = name + suf
                    sb[key] = wpool.tile([128, shape[1]], dt, name=f"sb_{key}")
                    nc.sync.dma_start(sb[key][:], dr[name][128 * half:128 * (half + 1), :])
            else:
                sb[name] = wpool.tile(shape, dt, name=f"sb_{name}")
                nc.sync.dma_start(sb[name][:], dr[name][:])

        X1 = wpool.tile([128, 9, T], BF16, name="X1")
        X2 = wpool.tile([128, 65, T], BF16, name="X2")
        WxF3 = wpool.tile([128, 32, H], BF16, name="WxF3")
        finB = wpool.tile([128, H, 32], BF16, name="finB")
        finT = wpool.tile([128, T, B], F32, name="finT")
        midA = wpool.tile([65, T, B], BF16, name="midA")
        midB = wpool.tile([64, T, B], BF16, name="midB")
        mid2T = wpool.tile([65, B, C2], BF16, name="mid2T")
        zeros128 = wpool.tile([128, 128], F32, name="zeros128")
        zeros64 = zeros128[:, 0:64]
        ones1 = wpool.tile([1, 64], F32, name="ones1")
        outsb = wpool.tile([B, 18], F32, name="outsb")

        nc.vector.memset(zeros128[:], 0.0)
        nc.vector.memset(ones1[:], 1.0)
        nc.vector.memset(mid2T[64:65, :, :], 1.0)
        nc.vector.memset(X2[64:128, 64, :], PAD_NEG)
        nc.vector.memset(X1[64:128, 8, :], PAD_NEG)
        # label -> mid2T[t, b, 128] and midB[63, t, b]
        nc.gpsimd.dma_start(mid2T[0:64, :, 128:129], dr["labelT"][:])
        nc.gpsimd.dma_start(midB[63:64, :, :], dr["labelT"][:])

        if only_stages != (1, 2, 3):
            # profiling variants: init tiles a skipped stage would have written
            nc.vector.memset(finT[:], 0.1)
            nc.vector.memset(finB[:], 0.1)
            nc.vector.memset(midA[:], 0.1)
            nc.vector.memset(midB[:], 0.1)
            nc.vector.memset(mid2T[:], 0.1)
            nc.vector.memset(X2[:], 0.1)
            nc.vector.memset(X1[:], 0.1)
            nc.vector.memset(WxF3[:], 0.1)
            nc.vector.memset(outsb[:], 0.0)

        # ---------- X1 build ----------
        with tc.tile_pool(name="xb1", space="PSUM", bufs=1) as xb:
            x1ps = xb.tile([128, 9, T], F32, name="x1ps")
            for c in range(NF):
                g, ch = (0, c) if c < 9 else (1, c - 9)
                rows = slice(g * 64, g * 64 + 64)
                nc.tensor.matmul(x1ps[rows, ch, :],
                                 sb["inpT"][:, c * B:(c + 1) * B],
                                 sb["Wi1R"][:], start=True, stop=True)
            nc.vector.tensor_copy(X1[0:64, :, :], x1ps[0:64, :, :])
            nc.scalar.copy(X1[64:128, 0:8, :], x1ps[64:128, 0:8, :])

        # ================= helpers =================
        def lstm_block(ps_gates, cB_old, cB_new, hB_new, pool):
            # Doubled-state LSTM: states are hS=2h, cS=2c; i/f/o pre-acts are
            # pre-halved in the weights so tanh(x) here equals 2*sigmoid-1.
            ta = pool.tile([64, 512], F32, name="ta", tag="ta", bufs=2)
            nc.scalar.activation(ta[:], ps_gates[:], AF.Tanh)
            u = pool.tile([64, 128], F32, name="u", tag="u", bufs=2)
            v2 = pool.tile([64, 128], F32, name="v2", tag="v2", bufs=2)
            # u = (tanh(i/2)+1)*tanh(g) = 2*sig(i)*tanh(g)
            nc.vector.scalar_tensor_tensor(u[:], ta[:, 0:128], 1.0,
                                           ta[:, 384:512], op0=OP.add, op1=OP.mult)
            # v = (tanh(f/2)+1)*cS = 4*sig(f)*c
            nc.vector.scalar_tensor_tensor(v2[:], ta[:, 128:256], 1.0,
                                           cB_old[:], op0=OP.add, op1=OP.mult)
            # cS_new = v/2 + u = 2*c_new
            nc.vector.scalar_tensor_tensor(cB_new[:], v2[:], 0.5,
                                           u[:], op0=OP.mult, op1=OP.add)
            tcel = pool.tile([64, 128], F32, name="tcel", tag="tcel", bufs=2)
            nc.scalar.activation(tcel[:], cB_new[:], AF.Tanh, scale=0.5)
            # hS_new = (tanh(o/2)+1)*tanh(c) = 2*h_new
            nc.vector.scalar_tensor_tensor(hB_new[:], ta[:, 256:384], 1.0,
                                           tcel[:], op0=OP.add, op1=OP.mult)


        def softmax_nomax(score, pool, ppool, nch, ptag="tps"):
            # score pad slots (if any) must already be ~-30 so exp ~ 0;
            # accum_out fuses the per-partition sum into the exp pass.
            ex = pool.tile([128, nch], F32, name="ex", tag="sm_ex", bufs=2)
            zs = pool.tile([128, 1], F32, name="zs", tag="sm_zs", bufs=2)
            nc.scalar.activation(ex[:], score[:], AF.Exp, accum_out=zs[:])
            zps = ppool.tile([128, 1], F32, name="zps", tag=ptag, bufs=3)
            nc.tensor.matmul(zps[:], sb["foldDup"][:], zs[:], start=True, stop=True)
            zr = pool.tile([128, 1], F32, name="zr", tag="sm_zr", bufs=2)
            nc.vector.reciprocal(zr[:], zps[:])
            a = pool.tile([128, nch], F32, name="a", tag="sm_a", bufs=2)
            nc.vector.tensor_scalar_mul(a[:], ex[:], zr[:])
            return a

        def tree_to(dst, src, pool, tag, nch, ntau):
            """sum src [128, nch, ntau] over tau into dst [128, nch] slice."""
            cur, n, lvl = src, ntau, 0
            while n > 2:
                n //= 2
                nxt = pool.tile([128, nch, n], BF16, name=f"{tag}_{lvl}",
                                tag=f"{tag}_{lvl}", bufs=1)
                nc.vector.tensor_add(nxt[:], cur[:, :, 0:n], cur[:, :, n:2 * n])
                cur, lvl = nxt, lvl + 1
            nc.vector.tensor_add(dst.unsqueeze(-1), cur[:, :, 0:1], cur[:, :, 1:2])

        def score_chunked(Xs, esb, vdup, nch, ntau, sp, tag, pad_neg=False):
            """returns score [128, nch] bf16; chunks over ch for engine overlap."""
            score = sp.tile([128, nch], BF16, name="score", tag=f"{tag}_score",
                            bufs=2)
            half = (nch + 1) // 2
            for lo, hi in ((0, half), (half, nch)):
                w = hi - lo
                scA = sp.tile([128, w, ntau], BF16, name="scA",
                              tag=f"{tag}_scA{lo}", bufs=1)
                nc.vector.tensor_add(scA[:], Xs[:, lo:hi, :],
                                     esb[:].unsqueeze(1).broadcast_to([128, w, ntau]))
                scT = sp.tile([128, w, ntau], BF16, name="scT",
                              tag=f"{tag}_scT{lo}", bufs=1)
                nc.scalar.activation(scT[:], scA[:], AF.Tanh)
                scM = sp.tile([128, w, ntau], BF16, name="scM",
                              tag=f"{tag}_scM{lo}", bufs=1)
                nc.vector.tensor_mul(scM[:], scT[:],
                                     vdup[:].unsqueeze(1).broadcast_to([128, w, ntau]))
                tree_to(score[:, lo:hi], scM, sp, f"{tag}_tr{lo}", w, ntau)
            if pad_neg:
                # kill the (g=1, ch=nch-1) pad slot before exp
                nc.vector.memset(score[64:128, nch - 1:nch], -30.0)
            return score

        # ================= encoder step =================
        def enc_step(t, stage, sp, pp, st):
            if stage == 1:
                Xs, vdup, WeRa, WeRb = X1, sb["vdup1"], sb["We1Ra"], sb["We1Rb"]
                nch = 9
                GH, GXA, GXB = sb["G1H"], sb["G1XA"], sb["G1XB"]
            else:
                Xs, vdup, WeRa, WeRb = X2, sb["vdup2"], sb["We2Ra"], sb["We2Rb"]
                nch = 65
                GH, GXA, GXB = sb["G2H"], sb["G2XA"], sb["G2XB"]
            hT_old, cT_old, cB_old = st["hT"], st["cT"], st["cB"]

            eps = pp.tile([128, T], F32, name="eps", tag="eps", bufs=2)
            for gb in (0, 64):
                o = eps[gb:gb + 64, :]
                nc.tensor.matmul(o, hT_old[:], WeRa[:], start=True, stop=False)
                nc.tensor.matmul(o, cT_old[:], WeRb[:], start=False, stop=True)
            esb = sp.tile([128, T], BF16, name="esb", tag="esb", bufs=2)
            nc.scalar.copy(esb[:], eps[:])

            if skip_score:
                score = sp.tile([128, nch], BF16, name="score", tag="e_score", bufs=2)
                nc.vector.memset(score[:], 0.1)
            else:
                score = score_chunked(Xs, esb, vdup, nch, T, sp, "e", pad_neg=True)
            a = softmax_nomax(score, sp, pp, nch)

            aTA = pp.tile([nch if nch > 9 else 9, 64], F32, name="aTA", tag="tps", bufs=3)
            nc.tensor.transpose(aTA[:], a[0:64, 0:(9 if nch == 9 else nch)], sb["I64dup"][0:64, :])
            aTB = pp.tile([(nch - 1) if nch > 9 else 8, 64], F32, name="aTB", tag="tps", bufs=3)
            nc.tensor.transpose(aTB[:], a[64:128, 0:(8 if nch == 9 else nch - 1)], sb["I64dup"][64:128, :])

            if stage == 1:
                xA = sp.tile([9, 64], F32, name="x1A", tag="xA", bufs=2)
                nc.vector.tensor_mul(xA[:],
                                     sb["inpCTA"][:, t * B:(t + 1) * B], aTA[:])
                xB = sp.tile([8, 64], F32, name="x1B", tag="xB", bufs=2)
                nc.vector.tensor_mul(xB[:], sb["inpCTB"][:, t * B:(t + 1) * B], aTB[:])
                brow = sb["b1row"]
            else:
                xA = sp.tile([65, 64], F32, name="x2A", tag="xA", bufs=2)
                nc.vector.tensor_mul(xA[:], midA[:, t, :], aTA[:])
                xB = sp.tile([64, 64], F32, name="x2B", tag="xB", bufs=2)
                nc.vector.tensor_mul(xB[:], midB[:, t, :], aTB[:])
                brow = sb["b2row"]

            gps = pp.tile([64, 512], F32, name="gps", tag="gps", bufs=2)
            nc.tensor.matmul(gps[:], ones1[:], brow[:], start=True, stop=False)
            nc.tensor.matmul(gps[:], hT_old[:], GH[:], start=False, stop=False)
            nc.tensor.matmul(gps[:], xA[:], GXA[:], start=False, stop=False)
            nc.tensor.matmul(gps[:], xB[:], GXB[:], start=False, stop=True)

            cB_new = sp.tile([64, 128], F32, name="cB", tag="cB", bufs=2)
            hB_new = sp.tile([64, 128], F32, name="hB", tag="hB", bufs=2)
            lstm_block(gps, cB_old, cB_new, hB_new, sp)

            hTps = pp.tile([128, 64], F32, name="hTps", tag="tps", bufs=3)
            nc.tensor.transpose(hTps[:], hB_new[:], sb["I64dup"][0:64, :])
            cTps = pp.tile([128, 64], F32, name="cTps", tag="tps", bufs=3)
            nc.tensor.transpose(cTps[:], cB_new[:], sb["I64dup"][0:64, :])
            cT_sb = sp.tile([128, 64], F32, name="cT_sb", tag="cT", bufs=2)
            nc.scalar.copy(cT_sb[:], cTps[:])

            if stage == 1:
                hT_sb = sp.tile([128, 64], F32, name="hT_sb", tag="hT", bufs=2)
                nc.vector.tensor_copy(hT_sb[:], hTps[:])
                nc.vector.tensor_copy(midA[:, t, :], hTps[0:65, :])
                shps = pp.tile([63, 64], F32, name="shps", tag="tps", bufs=3)
                nc.tensor.transpose(shps[:], hB_new[:, 65:128], sb["I64dup"][0:64, :])
                nc.scalar.copy(midB[0:63, t, :], shps[:])
                hbf = sp.tile([64, 128], BF16, name="hbf", tag="hbf", bufs=2)
                nc.scalar.copy(hbf[:], hB_new[:])
                nc.sync.dma_start(mid2T[t:t + 1, :, 0:128], hbf[:])
                st["hT"] = hT_sb
            else:
                nc.vector.tensor_copy(finT[:, t, :], hTps[:])
                g, sl = divmod(t, 32)
                if g == 0:
                    nc.scalar.copy(finB[0:64, :, sl], hB_new[:].unsqueeze(-1))
                else:
                    shf = pp.tile([128, 128], F32, name="shf", tag="gps", bufs=2)
                    nc.tensor.matmul(shf[64:128, :], sb["I64dup"][0:64, :],
                                     hB_new[:], start=True, stop=True)
                    nc.scalar.copy(finB[64:128, :, sl], shf[64:128, :].unsqueeze(-1))
                st["hT"] = finT[:, t, :]
            st["cT"], st["cB"] = cT_sb, cB_new

        # ---------- stage 1 ----------
        with tc.tile_pool(name="s1sp", bufs=2) as sp, \
             tc.tile_pool(name="s1pp", space="PSUM", bufs=2) as pp:
            st = {"hT": zeros64, "cT": zeros64, "cB": zeros128[0:64, :]}
            for t in range(T if 1 in only_stages else 0):
                enc_step(t, 1, sp, pp, st)

        # ---------- X2 build ----------
        with tc.tile_pool(name="xb2", space="PSUM", bufs=2) as xb2:
            for r in range(4):
                x2ps = xb2.tile([128, 16, T], F32, name="x2ps", tag="x2ps", bufs=2)
                for k in range(16):
                    ch = r * 16 + k
                    nc.tensor.matmul(x2ps[0:64, k, :], mid2T[:, :, ch],
                                     sb["Wi2R"][:], start=True, stop=True)
                    nc.tensor.matmul(x2ps[64:128, k, :], mid2T[:, :, 65 + ch],
                                     sb["Wi2R"][:], start=True, stop=True)
                nc.vector.tensor_copy(X2[:, r * 16:(r + 1) * 16, :], x2ps[:])
            x2ps2 = xb2.tile([64, T], F32, name="x2ps2", tag="x2ps2", bufs=1)
            nc.tensor.matmul(x2ps2[:], mid2T[:, :, 64], sb["Wi2R"][:],
                             start=True, stop=True)
            nc.vector.tensor_copy(X2[0:64, 64, :], x2ps2[:])

        # ---------- stage 2 ----------
        with tc.tile_pool(name="s2sp", bufs=2) as sp, \
             tc.tile_pool(name="s2pp", space="PSUM", bufs=2) as pp:
            st = {"hT": zeros64, "cT": zeros64, "cB": zeros128[0:64, :]}
            for t in range(T if 2 in only_stages else 0):
                enc_step(t, 2, sp, pp, st)

        # ---------- WxF build ----------
        with tc.tile_pool(name="wxb", space="PSUM", bufs=2) as wb:
            for r in range(16):
                g0, sl0 = divmod(r * 4, 32)
                rows = slice(g0 * 64, g0 * 64 + 64)
                wxps = wb.tile([128, 4, H], F32, name="wxps", tag="wxps", bufs=2)
                for j in range(4):
                    nc.tensor.matmul(wxps[rows, j, :], finT[:, r * 4 + j, :],
                                     sb["WxR"][:], start=True, stop=True)
                if r % 2 == 0:
                    nc.vector.tensor_copy(WxF3[rows, sl0:sl0 + 4, :], wxps[rows, :, :])
                else:
                    nc.scalar.copy(WxF3[rows, sl0:sl0 + 4, :], wxps[rows, :, :])

        # ---------- stage 3 ----------
        with tc.tile_pool(name="s3sp", bufs=2) as sp, \
             tc.tile_pool(name="s3pp", space="PSUM", bufs=2) as pp:
            outps = pp.tile([64, 18], F32, name="outps", bufs=1) if 3 in only_stages else None
            hT_old, cT_old = zeros64, zeros64
            cB_old = zeros128[0:64, :]
            for t in range(TD if 3 in only_stages else 0):
                eps = pp.tile([128, H], F32, name="e3ps", tag="eps3", bufs=2)
                for gb in (0, 64):
                    o = eps[gb:gb + 64, :]
                    nc.tensor.matmul(o, ones1[:], sb["Wxb"][:], start=True, stop=False)
                    nc.tensor.matmul(o, hT_old[:], sb["WhRa"][:],
                                     start=False, stop=False)
                    nc.tensor.matmul(o, cT_old[:], sb["WhRb"][:],
                                     start=False, stop=True)
                esb = sp.tile([128, H], BF16, name="e3sb", tag="esb3", bufs=2)
                nc.scalar.copy(esb[:], eps[:])

                if skip_score:
                    score = sp.tile([128, 32], BF16, name="score", tag="d_score", bufs=2)
                    nc.vector.memset(score[:], 0.1)
                else:
                    score = score_chunked(WxF3, esb, sb["vdup3"], 32, H, sp, "d")
                a = softmax_nomax(score, sp, pp, 32, ptag="tps3")
                abf = sp.tile([128, 32], BF16, name="abf", tag="abf", bufs=2)
                nc.vector.tensor_copy(abf[:], a[:])

                uu = sp.tile([128, H], BF16, name="uu", tag="uu", bufs=2)
                for lo, hi in ((0, 64), (64, H)):
                    ym = sp.tile([128, hi - lo, 32], BF16, name="ym",
                                 tag=f"ym{lo}", bufs=1)
                    nc.vector.tensor_mul(ym[:], finB[:, lo:hi, :],
                                         abf[:].unsqueeze(1).broadcast_to([128, hi - lo, 32]))
                    tree_to(uu[:, lo:hi], ym, sp, f"ctr{lo}", hi - lo, 32)
                dinps = pp.tile([64, H], F32, name="dinps", tag="tps3", bufs=3)
                nc.tensor.matmul(dinps[:], sb["foldLbf"][:], uu[:],
                                 start=True, stop=True)
                dinsb = sp.tile([64, H], F32, name="dinsb", tag="dinsb", bufs=2)
                nc.vector.tensor_copy(dinsb[:], dinps[:])
                dTps = pp.tile([128, 64], F32, name="dTps", tag="tps3", bufs=3)
                nc.tensor.transpose(dTps[:], dinsb[:], sb["I64dup"][0:64, :])
                dinT = sp.tile([128, 64], F32, name="dinT", tag="dinT", bufs=2)
                nc.vector.tensor_copy(dinT[:], dTps[:])

                gps = pp.tile([64, 512], F32, name="g3ps", tag="g3ps", bufs=2)
                nc.tensor.matmul(gps[:], ones1[:], sb["bdrow"][:], start=True, stop=False)
                nc.tensor.matmul(gps[:], hT_old[:], sb["GdH"][:], start=False, stop=False)
                nc.tensor.matmul(gps[:], dinT[:], sb["GdX"][:], start=False, stop=True)

                cB_new = sp.tile([64, 128], F32, name="c3B", tag="c3B", bufs=2)
                hB_new = sp.tile([64, 128], F32, name="h3B", tag="h3B", bufs=2)
                lstm_block(gps, cB_old, cB_new, hB_new, sp)
                cB_old = cB_new

                hTps = pp.tile([128, 64], F32, name="h3Tps", tag="tps3", bufs=3)
                nc.tensor.transpose(hTps[:], hB_new[:], sb["I64dup"][0:64, :])
                cTps = pp.tile([128, 64], F32, name="c3Tps", tag="tps3", bufs=3)
                nc.tensor.transpose(cTps[:], cB_new[:], sb["I64dup"][0:64, :])
                hT_sb = sp.tile([128, 64], F32, name="h3T", tag="h3T", bufs=2)
                nc.vector.tensor_copy(hT_sb[:], hTps[:])
                cT_sb = sp.tile([128, 64], F32, name="c3T", tag="c3T", bufs=2)
                nc.vector.tensor_copy(cT_sb[:], cTps[:])
                hT_old, cT_old = hT_sb, cT_sb

                if t >= TD - 18:
                    j = t - (TD - 18)
                    nc.tensor.matmul(outps[:, j:j + 1], hT_sb[:], sb["regw"][:],
                                     start=True, stop=True)

            if 3 in only_stages:
                nc.vector.tensor_copy(outsb[:], outps[:])
            nc.sync.dma_start(out_d[:], outsb[:])

        wpool.release()

    nc.compile()
    return nc


_NC_CACHE = {}


def kernel(**inputs):
    if "nc" not in _NC_CACHE:
        _NC_CACHE["nc"] = build_nc()
    nc = _NC_CACHE["nc"]
    w = prep_weights({k: np.asarray(v) for k, v in inputs.items()})
    in_maps = []
    for core in range(N_CORES):
        m = dict(w)
        m.update(prep_core_inputs(inputs, core))
        in_maps.append(m)
    res = run_bass_kernel_spmd(nc, in_maps, list(range(N_CORES)))
    out = np.concatenate([res.results[c]["out"] for c in range(N_CORES)], axis=0)
    out = out + np.asarray(inputs["reg_b"])[0]
    return out.astype(np.float32)


# revision 26
# speedup vs baseline: 166.5593x; 166.5593x over previous
# DSTP-RNN Trainium2 kernel: 8-core pure data parallel (batch 512 -> 64/core).
#
# Restructuring summary (validated numerically, rel-l2 ~7e-4 vs fp32 ref):
#  - "Score" tensors are b-major: partitions = (g, b) with g in {0,1} a
#    channel-group split, b = 64 local batch rows; free dims = (ch, tau).
#  - Per-step attention score: DVE broadcast-add of e, ACT tanh, DVE mul by
#    replicated v, DVE pairwise tree-reduce over tau (all bf16).
#  - Softmax without max-subtraction (scores are small); channel-group fold
#    and per-partition normalizer duplication via tiny PE matmuls.
#  - LSTM gates in b-major psum [64, 4H] via 3 matmuls with stationary
#    activations; gate order host-permuted to [i,f,o | g]; biases folded via
#    an appended ones-row on the stationary operand (or a K=1 init matmul).
#  - All cross-partition movement via PE (transpose matmuls with identity,
#    fold/dup matmuls with 0/1 matrices); DVE/ACT stay lane-aligned.
import numpy as np
import ml_dtypes

import concourse.bacc as bacc
import concourse.mybir as mybir
import concourse.tile as tile
from concourse.bass_utils import run_bass_kernel_spmd

F32 = mybir.dt.float32
BF16 = mybir.dt.bfloat16
AX = mybir.AxisListType
OP = mybir.AluOpType
AF = mybir.ActivationFunctionType

N_CORES = 8
B = 64      # batch per core
T = 64      # encoder length
H = 128
TD = 24     # decoder steps (T_DEC + 6)
NF = 17     # driving series count
C2 = 129    # stage-2 channels (H + label)
COLS = np.array(list(range(14)) + list(range(15, 18)))
PAD_NEG = -20.0   # pad channel fill (tanh -> -1; excluded from softmax sums)


def _perm_cols(w):
    # torch gate order (i,f,g,o) -> (i,f,o,g): sigmoid block contiguous
    i, f, g, o = np.split(w, 4, axis=-1)
    return np.concatenate([i, f, o, g], axis=-1)


def _bf(x):
    return np.ascontiguousarray(np.asarray(x).astype(ml_dtypes.bfloat16))


def _f32(x):
    return np.ascontiguousarray(np.asarray(x).astype(np.float32))


def prep_weights(inp):
    w = {}
    w["Wi1R"] = _bf(np.concatenate([inp["Wi_w"].T, inp["Wi_b"][None, :]], 0))
    w["Wi2R"] = _bf(np.concatenate([inp["Wi2_w"].T * 0.5, inp["Wi2_b"][None, :]], 0))
    w["We1R"] = _f32(inp["We_w"].T * 0.5)
    w["We2R"] = _f32(inp["We2_w"].T * 0.5)
    w["WhR"] = _f32(inp["Wh_w"].T * 0.5)
    w["WxR"] = _f32(inp["Wx_w"].T * 0.5)
    w["Wxb"] = _f32(inp["Wx_b"][None, :])

    # ISO: sigmoid gates computed as tanh(x/2) -> pre-scale i,f,o cols by 0.5.
    # States are stored doubled (hS=2h, cS=2c), so weight blocks consuming
    # h/c/mid/din get an extra 0.5.
    ISO = np.concatenate([0.5 * np.ones(384), np.ones(128)]).astype(np.float32)
    g1x = _perm_cols(inp["Wih1"].T) * ISO
    b1 = _perm_cols((inp["bih1"] + inp["bhh1"])[None, :]) * ISO
    w["G1XA"] = _f32(g1x[0:9])
    w["b1row"] = _f32(b1)
    w["G1XB"] = _f32(g1x[9:17])
    w["G1H"] = _f32(_perm_cols(inp["Whh1"].T) * ISO * 0.5)

    g2x = _perm_cols(inp["Wih2"].T) * ISO * 0.5
    b2 = _perm_cols((inp["bih2"] + inp["bhh2"])[None, :]) * ISO
    w["G2XA"] = _f32(g2x[0:65])
    w["b2row"] = _f32(b2)
    w["G2XB"] = _f32(g2x[65:129])
    w["G2H"] = _f32(_perm_cols(inp["Whh2"].T) * ISO * 0.5)

    w["GdX"] = _f32(_perm_cols(inp["Wihd"].T) * ISO * 0.5)
    w["GdH"] = _f32(_perm_cols(inp["Whhd"].T) * ISO * 0.5)
    w["bdrow"] = _f32(_perm_cols((inp["bihd"] + inp["bhhd"])[None, :]) * ISO)

    w["vdup1"] = _bf(np.broadcast_to(inp["Vd_w"][0][None, :], (128, T)))
    w["vdup2"] = _bf(np.broadcast_to(inp["Vd2_w"][0][None, :], (128, T)))
    w["vdup3"] = _bf(np.broadcast_to(inp["V_w"][0][None, :], (128, H)))
    w["regw"] = _f32(inp["reg_w"][0][:, None] * 0.5)

    eye = np.eye(64, dtype=np.float32)
    w["I64dup"] = _f32(np.concatenate([eye, eye], 0))
    foldL = np.zeros((128, 64), np.float32)
    for p in range(128):
        foldL[p, p % 64] = 1.0
    w["foldLbf"] = _bf(foldL)
    foldDup = (np.arange(128)[:, None] % 64 == np.arange(128)[None, :] % 64)
    w["foldDup"] = _f32(foldDup.astype(np.float32))
    return w


def prep_core_inputs(inp, core):
    b0, b1 = core * B, (core + 1) * B
    x = np.asarray(inp["input_p_q"])[b0:b1, :T, :][:, :, COLS]   # [64,64,17]
    lab = np.asarray(inp["label_p"])[b0:b1, :T]                  # [64,64]
    d = {}
    inpT = np.ones((65, NF * B), np.float32)
    inpT[:64] = x.transpose(1, 2, 0).reshape(64, NF * B)         # [t, (c,b)]
    d["inpT"] = _bf(inpT)
    ct = x.transpose(2, 1, 0).reshape(NF, T * B)                 # [c, (t,b)]
    d["inpCTA"] = _bf(ct[0:9])
    d["inpCTB"] = _bf(ct[9:17])
    d["labelT"] = _f32(lab.T * 2.0)                                    # [t, b]
    return d


DRAM_SPECS = {
    "inpT": ([65, NF * B], BF16), "inpCTA": ([9, T * B], BF16),
    "inpCTB": ([8, T * B], BF16), "labelT": ([T, B], F32),
    "Wi1R": ([65, 64], BF16), "Wi2R": ([65, 64], BF16),
    "We1R": ([256, 64], F32), "We2R": ([256, 64], F32),
    "WhR": ([256, 128], F32), "WxR": ([128, 128], F32), "Wxb": ([1, 128], F32),
    "G1XA": ([9, 512], F32), "b1row": ([1, 512], F32), "G1XB": ([8, 512], F32), "G1H": ([128, 512], F32),
    "G2XA": ([65, 512], F32), "b2row": ([1, 512], F32), "G2XB": ([64, 512], F32), "G2H": ([128, 512], F32),
    "GdX": ([128, 512], F32), "GdH": ([128, 512], F32), "bdrow": ([1, 512], F32),
    "vdup1": ([128, T], BF16), "vdup2": ([128, T], BF16), "vdup3": ([128, H], BF16),
    "regw": ([128, 1], F32), "I64dup": ([128, 64], F32),
    "foldLbf": ([128, 64], BF16), "foldDup": ([128, 128], F32),
}


def build_nc(num_devices=N_CORES, skip_score=False, skip_tail=False, only_stages=(1, 2, 3)):
    nc = bacc.Bacc("TRN2", target_bir_lowering=False, debug=False,
                   num_devices=num_devices)
    dr = {}
    for name, (shape, dt) in DRAM_SPECS.items():
        dr[name] = nc.dram_tensor(name, shape, dt, kind="ExternalInput").ap()
    out_d = nc.dram_tensor("out", [B, 18], F32, kind="ExternalOutput").ap()

    with tile.TileContext(nc) as tc:
        # ---------- persistent SBUF ----------
        wpool = tc.alloc_tile_pool(name="wpool", bufs=1)
        sb = {}
        for name, (shape, dt) in DRAM_SPECS.items():
            if shape[0] > 128:
                assert shape[0] == 256
                for half, suf in ((0, "a"), (1, "b")):
                    key = name + suf
                    sb[key] = wpool.tile([128, shape[1]], dt, name=f"sb_{key}")
                    nc.sync.dma_start(sb[key][:], dr[name][128 * half:128 * (half + 1), :])
            else:
                sb[name] = wpool.tile(shape, dt, name=f"sb_{name}")
                nc.sync.dma_start(sb[name][:], dr[name][:])

        X1 = wpool.tile([128, 9, T], BF16, name="X1")
        X2 = wpool.tile([128, 65, T], BF16, name="X2")
        WxF3 = wpool.tile([128, 32, H], BF16, name="WxF3")
        finB = wpool.tile([128, H, 32], BF16, name="finB")
        finT = wpool.tile([128, T, B], F32, name="finT")
        midA = wpool.tile([65, T, B], BF16, name="midA")
        midB = wpool.tile([64, T, B], BF16, name="midB")
        mid2T = wpool.tile([65, B, C2], BF16, name="mid2T")
        zeros128 = wpool.tile([128, 128], F32, name="zeros128")
        zeros64 = zeros128[:, 0:64]
        ones1 = wpool.tile([1, 64], F32, name="ones1")
        outsb = wpool.tile([B, 18], F32, name="outsb")

        nc.vector.memset(zeros128[:], 0.0)
        nc.vector.memset(ones1[:], 1.0)
        nc.vector.memset(mid2T[64:65, :, :], 1.0)
        nc.vector.memset(X2[64:128, 64, :], PAD_NEG)
        nc.vector.memset(X1[64:128, 8, :], PAD_NEG)
        # label -> mid2T[t, b, 128] and midB[63, t, b]
        nc.gpsimd.dma_start(mid2T[0:64, :, 128:129], dr["labelT"][:])
        nc.gpsimd.dma_start(midB[63:64, :, :], dr["labelT"][:])

        if only_stages != (1, 2, 3):
            # profiling variants: init tiles a skipped stage would have written
            nc.vector.memset(finT[:], 0.1)
            nc.vector.memset(finB[:], 0.1)
            nc.vector.memset(midA[:], 0.1)
            nc.vector.memset(midB[:], 0.1)
            nc.vector.memset(mid2T[:], 0.1)
            nc.vector.memset(X2[:], 0.1)
            nc.vector.memset(X1[:], 0.1)
            nc.vector.memset(WxF3[:], 0.1)
            nc.vector.memset(outsb[:], 0.0)

        # ---------- X1 build ----------
        with tc.tile_pool(name="xb1", space="PSUM", bufs=1) as xb:
            x1ps = xb.tile([128, 9, T], F32, name="x1ps")
            for c in range(NF):
                g, ch = (0, c) if c < 9 else (1, c - 9)
                rows = slice(g * 64, g * 64 + 64)
                nc.tensor.matmul(x1ps[rows, ch, :],
                                 sb["inpT"][:, c * B:(c + 1) * B],
                                 sb["Wi1R"][:], start=True, stop=True)
            nc.vector.tensor_copy(X1[0:64, :, :], x1ps[0:64, :, :])
            nc.scalar.copy(X1[64:128, 0:8, :], x1ps[64:128, 0:8, :])

        # ================= helpers =================
        def lstm_block(ps_gates, cB_old, cB_new, hB_new, pool):
            # Doubled-state LSTM: states are hS=2h, cS=2c; i/f/o pre-acts are
            # pre-halved in the weights so tanh(x) here equals 2*sigmoid-1.
            ta = pool.tile([64, 512], F32, name="ta", tag="ta", bufs=2)
            nc.scalar.activation(ta[:], ps_gates[:], AF.Tanh)
            u = pool.tile([64, 128], F32, name="u", tag="u", bufs=2)
            v2 = pool.tile([64, 128], F32, name="v2", tag="v2", bufs=2)
            # u = (tanh(i/2)+1)*tanh(g) = 2*sig(i)*tanh(g)
            nc.vector.scalar_tensor_tensor(u[:], ta[:, 0:128], 1.0,
                                           ta[:, 384:512], op0=OP.add, op1=OP.mult)
            # v = (tanh(f/2)+1)*cS = 4*sig(f)*c
            nc.vector.scalar_tensor_tensor(v2[:], ta[:, 128:256], 1.0,
                                           cB_old[:], op0=OP.add, op1=OP.mult)
            # cS_new = v/2 + u = 2*c_new
            nc.vector.scalar_tensor_tensor(cB_new[:], v2[:], 0.5,
                                           u[:], op0=OP.mult, op1=OP.add)
            tcel = pool.tile([64, 128], F32, name="tcel", tag="tcel", bufs=2)
            nc.scalar.activation(tcel[:], cB_new[:], AF.Tanh, scale=0.5)
            # hS_new = (tanh(o/2)+1)*tanh(c) = 2*h_new
            nc.vector.scalar_tensor_tensor(hB_new[:], ta[:, 256:384], 1.0,
                                           tcel[:], op0=OP.add, op1=OP.mult)


        def softmax_nomax(score, pool, ppool, nch, ptag="tps"):
            # score pad slots (if any) must already be ~-30 so exp ~ 0;
            # accum_out fuses the per-partition sum into the exp pass.
            ex = pool.tile([128, nch], F32, name="ex", tag="sm_ex", bufs=2)
            zs = pool.tile([128, 1], F32, name="zs", tag="sm_zs", bufs=2)
            nc.scalar.activation(ex[:], score[:], AF.Exp, accum_out=zs[:])
            zps = ppool.tile([128, 1], F32, name="zps", tag=ptag,
                             bufs=4 if ptag == "tps" else 3)
            nc.tensor.matmul(zps[:], sb["foldDup"][:], zs[:], start=True, stop=True)
            zr = pool.tile([128, 1], F32, name="zr", tag="sm_zr", bufs=2)
            nc.vector.reciprocal(zr[:], zps[:])
            a = pool.tile([128, nch], F32, name="a", tag="sm_a", bufs=2)
            nc.vector.tensor_scalar_mul(a[:], ex[:], zr[:])
            return a

        def tree_to(dst, src, pool, tag, nch, ntau):
            """sum src [128, nch, ntau] over tau into dst [128, nch] slice."""
            nb = 1
            cur, n, lvl = src, ntau, 0
            while n > 2:
                n //= 2
                nxt = pool.tile([128, nch, n], BF16, name=f"{tag}_{lvl}",
                                tag=f"{tag}_{lvl}", bufs=nb)
                nc.vector.tensor_add(nxt[:], cur[:, :, 0:n], cur[:, :, n:2 * n])
                cur, lvl = nxt, lvl + 1
            nc.vector.tensor_add(dst.unsqueeze(-1), cur[:, :, 0:1], cur[:, :, 1:2])

        def score_chunked(Xs, esb, vdup, nch, ntau, sp, tag, pad_neg=False,
                          nchunks=2):
            """returns score [128, nch] bf16; chunks over ch for engine overlap."""
            score = sp.tile([128, nch], BF16, name="score", tag=f"{tag}_score",
                            bufs=2)
            if nchunks == 1:
                bounds = ((0, nch),)
            elif nchunks == 2:
                half = (nch + 1) // 2
                bounds = ((0, half), (half, nch))
            else:
                q = max(1, nch // nchunks)
                cuts = list(range(0, nch, q))
                bounds = tuple((lo, min(lo + q, nch)) for lo in cuts)
            for lo, hi in bounds:
                w = hi - lo
                nb = 1
                scA = sp.tile([128, w, ntau], BF16, name="scA",
                              tag=f"{tag}_scA{lo}", bufs=nb)
                nc.vector.tensor_add(scA[:], Xs[:, lo:hi, :],
                                     esb[:].unsqueeze(1).broadcast_to([128, w, ntau]))
                scT = sp.tile([128, w, ntau], BF16, name="scT",
                              tag=f"{tag}_scT{lo}", bufs=nb)
                nc.scalar.activation(scT[:], scA[:], AF.Tanh)
                scM = sp.tile([128, w, ntau], BF16, name="scM",
                              tag=f"{tag}_scM{lo}", bufs=nb)
                nc.vector.tensor_mul(scM[:], scT[:],
                                     vdup[:].unsqueeze(1).broadcast_to([128, w, ntau]))
                tree_to(score[:, lo:hi], scM, sp, f"{tag}_tr{lo}", w, ntau)
            if pad_neg:
                # kill the (g=1, ch=nch-1) pad slot before exp
                nc.vector.memset(score[64:128, nch - 1:nch], -30.0)
            return score

        # ================= encoder step =================
        def enc_step(t, stage, sp, pp, st):
            if stage == 1:
                Xs, vdup, WeRa, WeRb = X1, sb["vdup1"], sb["We1Ra"], sb["We1Rb"]
                nch = 9
                GH, GXA, GXB = sb["G1H"], sb["G1XA"], sb["G1XB"]
            else:
                Xs, vdup, WeRa, WeRb = X2, sb["vdup2"], sb["We2Ra"], sb["We2Rb"]
                nch = 65
                GH, GXA, GXB = sb["G2H"], sb["G2XA"], sb["G2XB"]
            hT_old, cT_old, cB_old = st["hT"], st["cT"], st["cB"]

            eps = pp.tile([128, T], F32, name="eps", tag="eps", bufs=2)
            for gb in (0, 64):
                o = eps[gb:gb + 64, :]
                nc.tensor.matmul(o, hT_old[:], WeRa[:], start=True, stop=False)
                nc.tensor.matmul(o, cT_old[:], WeRb[:], start=False, stop=True)
            esb = sp.tile([128, T], BF16, name="esb", tag="esb", bufs=2)
            nc.scalar.copy(esb[:], eps[:])

            if skip_score:
                score = sp.tile([128, nch], BF16, name="score", tag="e_score", bufs=2)
                nc.vector.memset(score[:], 0.1)
            else:
                score = score_chunked(Xs, esb, vdup, nch, T, sp, "e", pad_neg=True,
                                      nchunks=2)
            a = softmax_nomax(score, sp, pp, nch)

            aTA = pp.tile([nch if nch > 9 else 9, 64], F32, name="aTA", tag="tps", bufs=4)
            nc.tensor.transpose(aTA[:], a[0:64, 0:(9 if nch == 9 else nch)], sb["I64dup"][0:64, :])
            aTB = pp.tile([(nch - 1) if nch > 9 else 8, 64], F32, name="aTB", tag="tps", bufs=4)
            nc.tensor.transpose(aTB[:], a[64:128, 0:(8 if nch == 9 else nch - 1)], sb["I64dup"][64:128, :])

            if stage == 1:
                xA = sp.tile([9, 64], F32, name="x1A", tag="xA", bufs=2)
                nc.vector.tensor_mul(xA[:],
                                     sb["inpCTA"][:, t * B:(t + 1) * B], aTA[:])
                xB = sp.tile([8, 64], F32, name="x1B", tag="xB", bufs=2)
                nc.vector.tensor_mul(xB[:], sb["inpCTB"][:, t * B:(t + 1) * B], aTB[:])
                brow = sb["b1row"]
            else:
                xA = sp.tile([65, 64], F32, name="x2A", tag="xA", bufs=2)
                nc.vector.tensor_mul(xA[:], midA[:, t, :], aTA[:])
                xB = sp.tile([64, 64], F32, name="x2B", tag="xB", bufs=2)
                nc.vector.tensor_mul(xB[:], midB[:, t, :], aTB[:])
                brow = sb["b2row"]

            gps = pp.tile([64, 512], F32, name="gps", tag="gps", bufs=2)
            nc.tensor.matmul(gps[:], ones1[:], brow[:], start=True, stop=False)
            nc.tensor.matmul(gps[:], hT_old[:], GH[:], start=False, stop=False)
            nc.tensor.matmul(gps[:], xA[:], GXA[:], start=False, stop=False)
            nc.tensor.matmul(gps[:], xB[:], GXB[:], start=False, stop=True)

            cB_new = sp.tile([64, 128], F32, name="cB", tag="cB", bufs=2)
            hB_new = sp.tile([64, 128], F32, name="hB", tag="hB", bufs=2)
            lstm_block(gps, cB_old, cB_new, hB_new, sp)

            hTps = pp.tile([128, 64], F32, name="hTps", tag="tps", bufs=4)
            nc.tensor.transpose(hTps[:], hB_new[:], sb["I64dup"][0:64, :])
            cTps = pp.tile([128, 64], F32, name="cTps", tag="tps", bufs=4)
            nc.tensor.transpose(cTps[:], cB_new[:], sb["I64dup"][0:64, :])
            cT_sb = sp.tile([128, 64], F32, name="cT_sb", tag="cT", bufs=2)
            nc.scalar.copy(cT_sb[:], cTps[:])

            if stage == 1:
                hT_sb = sp.tile([128, 64], F32, name="hT_sb", tag="hT", bufs=2)
                nc.vector.tensor_copy(hT_sb[:], hTps[:])
                nc.vector.tensor_copy(midA[:, t, :], hTps[0:65, :])
                shps = pp.tile([63, 64], F32, name="shps", tag="tps", bufs=4)
                nc.tensor.transpose(shps[:], hB_new[:, 65:128], sb["I64dup"][0:64, :])
                nc.scalar.copy(midB[0:63, t, :], shps[:])
                hbf = sp.tile([64, 128], BF16, name="hbf", tag="hbf", bufs=2)
                nc.scalar.copy(hbf[:], hB_new[:])
                nc.sync.dma_start(mid2T[t:t + 1, :, 0:128], hbf[:])
                st["hT"] = hT_sb
            else:
                nc.vector.tensor_copy(finT[:, t, :], hTps[:])
                g, sl = divmod(t, 32)
                if g == 0:
                    nc.scalar.copy(finB[0:64, :, sl], hB_new[:].unsqueeze(-1))
                else:
                    shf = pp.tile([128, 128], F32, name="shf", tag="gps", bufs=2)
                    nc.tensor.matmul(shf[64:128, :], sb["I64dup"][0:64, :],
                                     hB_new[:], start=True, stop=True)
                    nc.scalar.copy(finB[64:128, :, sl], shf[64:128, :].unsqueeze(-1))
                st["hT"] = finT[:, t, :]
            st["cT"], st["cB"] = cT_sb, cB_new

        # ---------- stage 1 ----------
        with tc.tile_pool(name="s1sp", bufs=2) as sp, \
             tc.tile_pool(name="s1pp", space="PSUM", bufs=2) as pp:
            st = {"hT": zeros64, "cT": zeros64, "cB": zeros128[0:64, :]}
            for t in range(T if 1 in only_stages else 0):
                enc_step(t, 1, sp, pp, st)

        # ---------- X2 build ----------
        with tc.tile_pool(name="xb2", space="PSUM", bufs=2) as xb2:
            for r in range(4):
                x2ps = xb2.tile([128, 16, T], F32, name="x2ps", tag="x2ps", bufs=2)
                for k in range(16):
                    ch = r * 16 + k
                    nc.tensor.matmul(x2ps[0:64, k, :], mid2T[:, :, ch],
                                     sb["Wi2R"][:], start=True, stop=True)
                    nc.tensor.matmul(x2ps[64:128, k, :], mid2T[:, :, 65 + ch],
                                     sb["Wi2R"][:], start=True, stop=True)
                nc.vector.tensor_copy(X2[:, r * 16:(r + 1) * 16, :], x2ps[:])
            x2ps2 = xb2.tile([64, T], F32, name="x2ps2", tag="x2ps2", bufs=1)
            nc.tensor.matmul(x2ps2[:], mid2T[:, :, 64], sb["Wi2R"][:],
                             start=True, stop=True)
            nc.vector.tensor_copy(X2[0:64, 64, :], x2ps2[:])

        # ---------- stage 2 ----------
        with tc.tile_pool(name="s2sp", bufs=2) as sp, \
             tc.tile_pool(name="s2pp", space="PSUM", bufs=2) as pp:
            st = {"hT": zeros64, "cT": zeros64, "cB": zeros128[0:64, :]}
            for t in range(T if 2 in only_stages else 0):
                enc_step(t, 2, sp, pp, st)

        # ---------- WxF build ----------
        with tc.tile_pool(name="wxb", space="PSUM", bufs=2) as wb:
            for r in range(16):
                g0, sl0 = divmod(r * 4, 32)
                rows = slice(g0 * 64, g0 * 64 + 64)
                wxps = wb.tile([128, 4, H], F32, name="wxps", tag="wxps", bufs=2)
                for j in range(4):
                    nc.tensor.matmul(wxps[rows, j, :], finT[:, r * 4 + j, :],
                                     sb["WxR"][:], start=True, stop=True)
                if r % 2 == 0:
                    nc.vector.tensor_copy(WxF3[rows, sl0:sl0 + 4, :], wxps[rows, :, :])
                else:
                    nc.scalar.copy(WxF3[rows, sl0:sl0 + 4, :], wxps[rows, :, :])

        # ---------- stage 3 ----------
        with tc.tile_pool(name="s3sp", bufs=2) as sp, \
             tc.tile_pool(name="s3pp", space="PSUM", bufs=2) as pp:
            outps = pp.tile([64, 18], F32, name="outps", bufs=1) if 3 in only_stages else None
            hT_old, cT_old = zeros64, zeros64
            cB_old = zeros128[0:64, :]
            for t in range(TD if 3 in only_stages else 0):
                eps = pp.tile([128, H], F32, name="e3ps", tag="eps3", bufs=2)
                for gb in (0, 64):
                    o = eps[gb:gb + 64, :]
                    nc.tensor.matmul(o, ones1[:], sb["Wxb"][:], start=True, stop=False)
                    nc.tensor.matmul(o, hT_old[:], sb["WhRa"][:],
                                     start=False, stop=False)
                    nc.tensor.matmul(o, cT_old[:], sb["WhRb"][:],
                                     start=False, stop=True)
                esb = sp.tile([128, H], BF16, name="e3sb", tag="esb3", bufs=2)
                nc.scalar.copy(esb[:], eps[:])

                if skip_score:
                    score = sp.tile([128, 32], BF16, name="score", tag="d_score", bufs=2)
                    nc.vector.memset(score[:], 0.1)
                else:
                    score = score_chunked(WxF3, esb, sb["vdup3"], 32, H, sp, "d")
                a = softmax_nomax(score, sp, pp, 32, ptag="tps3")
                abf = sp.tile([128, 32], BF16, name="abf", tag="abf", bufs=2)
                nc.vector.tensor_copy(abf[:], a[:])

                uu = sp.tile([128, H], BF16, name="uu", tag="uu", bufs=2)
                for lo, hi in ((0, 64), (64, H)):
                    ym = sp.tile([128, hi - lo, 32], BF16, name="ym",
                                 tag=f"ym{lo}", bufs=1)
                    nc.vector.tensor_mul(ym[:], finB[:, lo:hi, :],
                                         abf[:].unsqueeze(1).broadcast_to([128, hi - lo, 32]))
                    tree_to(uu[:, lo:hi], ym, sp, f"ctr{lo}", hi - lo, 32)
                dinps = pp.tile([64, H], F32, name="dinps", tag="tps3", bufs=3)
                nc.tensor.matmul(dinps[:], sb["foldLbf"][:], uu[:],
                                 start=True, stop=True)
                dinsb = sp.tile([64, H], F32, name="dinsb", tag="dinsb", bufs=2)
                nc.vector.tensor_copy(dinsb[:], dinps[:])
                dTps = pp.tile([128, 64], F32, name="dTps", tag="tps3", bufs=3)
                nc.tensor.transpose(dTps[:], dinsb[:], sb["I64dup"][0:64, :])
                dinT = sp.tile([128, 64], F32, name="dinT", tag="dinT", bufs=2)
                nc.vector.tensor_copy(dinT[:], dTps[:])

                gps = pp.tile([64, 512], F32, name="g3ps", tag="g3ps", bufs=2)
                nc.tensor.matmul(gps[:], ones1[:], sb["bdrow"][:], start=True, stop=False)
                nc.tensor.matmul(gps[:], hT_old[:], sb["GdH"][:], start=False, stop=False)
                nc.tensor.matmul(gps[:], dinT[:], sb["GdX"][:], start=False, stop=True)

                cB_new = sp.tile([64, 128], F32, name="c3B", tag="c3B", bufs=2)
                hB_new = sp.tile([64, 128], F32, name="h3B", tag="h3B", bufs=2)
                lstm_block(gps, cB_old, cB_new, hB_new, sp)
                cB_old = cB_new

                hTps = pp.tile([128, 64], F32, name="h3Tps", tag="tps3", bufs=3)
                nc.tensor.transpose(hTps[:], hB_new[:], sb["I64dup"][0:64, :])
                cTps = pp.tile([128, 64], F32, name="c3Tps", tag="tps3", bufs=3)
                nc.tensor.transpose(cTps[:], cB_new[:], sb["I64dup"][0:64, :])
                hT_sb = sp.tile([128, 64], F32, name="h3T", tag="h3T", bufs=2)
                nc.vector.tensor_copy(hT_sb[:], hTps[:])
                cT_sb = sp.tile([128, 64], F32, name="c3T", tag="c3T", bufs=2)
                nc.vector.tensor_copy(cT_sb[:], cTps[:])
                hT_old, cT_old = hT_sb, cT_sb

                if t >= TD - 18:
                    j = t - (TD - 18)
                    nc.tensor.matmul(outps[:, j:j + 1], hT_sb[:], sb["regw"][:],
                                     start=True, stop=True)

            if 3 in only_stages:
                nc.vector.tensor_copy(outsb[:], outps[:])
            nc.sync.dma_start(out_d[:], outsb[:])

        wpool.release()

    nc.compile()
    return nc


_NC_CACHE = {}


def kernel(**inputs):
    if "nc" not in _NC_CACHE:
        _NC_CACHE["nc"] = build_nc()
    nc = _NC_CACHE["nc"]
    w = prep_weights({k: np.asarray(v) for k, v in inputs.items()})
    in_maps = []
    for core in range(N_CORES):
        m = dict(w)
        m.update(prep_core_inputs(inputs, core))
        in_maps.append(m)
    res = run_bass_kernel_spmd(nc, in_maps, list(range(N_CORES)))
    out = np.concatenate([res.results[c]["out"] for c in range(N_CORES)], axis=0)
    out = out + np.asarray(inputs["reg_b"])[0]
    return out.astype(np.float32)


# revision 28
# speedup vs baseline: 478.3203x; 2.8718x over previous
# DSTP-RNN Trainium2 kernel: 8-core pure data parallel (batch 512 -> 64/core).
#
# Restructuring summary (validated numerically, rel-l2 ~7e-4 vs fp32 ref):
#  - "Score" tensors are b-major: partitions = (g, b) with g in {0,1} a
#    channel-group split, b = 64 local batch rows; free dims = (ch, tau).
#  - Per-step attention score: DVE broadcast-add of e, ACT tanh, DVE mul by
#    replicated v, DVE pairwise tree-reduce over tau (all bf16).
#  - Softmax without max-subtraction (scores are small); channel-group fold
#    and per-partition normalizer duplication via tiny PE matmuls.
#  - LSTM gates in b-major psum [64, 4H] via 3 matmuls with stationary
#    activations; gate order host-permuted to [i,f,o | g]; biases folded via
#    an appended ones-row on the stationary operand (or a K=1 init matmul).
#  - All cross-partition movement via PE (transpose matmuls with identity,
#    fold/dup matmuls with 0/1 matrices); DVE/ACT stay lane-aligned.
import numpy as np
import ml_dtypes

import concourse.bacc as bacc
import concourse.mybir as mybir
import concourse.tile as tile
from concourse.bass_utils import run_bass_kernel_spmd

F32 = mybir.dt.float32
BF16 = mybir.dt.bfloat16
AX = mybir.AxisListType
OP = mybir.AluOpType
AF = mybir.ActivationFunctionType

N_CORES = 8
B = 64      # batch per core
T = 64      # encoder length
H = 128
TD = 24     # decoder steps (T_DEC + 6)
NF = 17     # driving series count
C2 = 129    # stage-2 channels (H + label)
COLS = np.array(list(range(14)) + list(range(15, 18)))
PAD_NEG = -20.0   # pad channel fill (tanh -> -1; excluded from softmax sums)


def _perm_cols(w):
    # torch gate order (i,f,g,o) -> (i,f,o,g): sigmoid block contiguous
    i, f, g, o = np.split(w, 4, axis=-1)
    return np.concatenate([i, f, o, g], axis=-1)


def _bf(x):
    return np.ascontiguousarray(np.asarray(x).astype(ml_dtypes.bfloat16))


def _f32(x):
    return np.ascontiguousarray(np.asarray(x).astype(np.float32))


def prep_weights(inp):
    w = {}
    w["Wi1R"] = _bf(np.concatenate([inp["Wi_w"].T, inp["Wi_b"][None, :]], 0))
    w["Wi2R"] = _bf(np.concatenate([inp["Wi2_w"].T * 0.5, inp["Wi2_b"][None, :]], 0))
    w["We1R"] = _f32(inp["We_w"].T * 0.5)
    w["We2R"] = _f32(inp["We2_w"].T * 0.5)
    w["WhR"] = _f32(inp["Wh_w"].T * 0.5)
    w["WxR"] = _f32(inp["Wx_w"].T * 0.5)
    w["Wxb"] = _f32(inp["Wx_b"][None, :])

    # ISO: sigmoid gates computed as tanh(x/2) -> pre-scale i,f,o cols by 0.5.
    # States are stored doubled (hS=2h, cS=2c), so weight blocks consuming
    # h/c/mid/din get an extra 0.5.
    ISO = np.concatenate([0.5 * np.ones(384), np.ones(128)]).astype(np.float32)
    g1x = _perm_cols(inp["Wih1"].T) * ISO
    b1 = _perm_cols((inp["bih1"] + inp["bhh1"])[None, :]) * ISO
    w["G1XA"] = _f32(g1x[0:9])
    w["b1row"] = _f32(b1)
    w["G1XB"] = _f32(g1x[9:17])
    w["G1H"] = _f32(_perm_cols(inp["Whh1"].T) * ISO * 0.5)

    g2x = _perm_cols(inp["Wih2"].T) * ISO * 0.5
    b2 = _perm_cols((inp["bih2"] + inp["bhh2"])[None, :]) * ISO
    w["G2XA"] = _f32(g2x[0:65])
    w["b2row"] = _f32(b2)
    w["G2XB"] = _f32(g2x[65:129])
    w["G2H"] = _f32(_perm_cols(inp["Whh2"].T) * ISO * 0.5)

    w["GdX"] = _f32(_perm_cols(inp["Wihd"].T) * ISO * 0.5)
    w["GdH"] = _f32(_perm_cols(inp["Whhd"].T) * ISO * 0.5)
    w["bdrow"] = _f32(_perm_cols((inp["bihd"] + inp["bhhd"])[None, :]) * ISO)

    w["vdup1"] = _bf(np.broadcast_to(inp["Vd_w"][0][None, :], (128, T)))
    w["vdup2"] = _bf(np.broadcast_to(inp["Vd2_w"][0][None, :], (128, T)))
    w["vdup3"] = _bf(np.broadcast_to(inp["V_w"][0][None, :], (128, H)))
    w["regw"] = _f32(inp["reg_w"][0][:, None] * 0.5)

    eye = np.eye(64, dtype=np.float32)
    w["I64dup"] = _f32(np.concatenate([eye, eye], 0))
    foldL = np.zeros((128, 64), np.float32)
    for p in range(128):
        foldL[p, p % 64] = 1.0
    w["foldLbf"] = _bf(foldL)
    foldDup = (np.arange(128)[:, None] % 64 == np.arange(128)[None, :] % 64)
    w["foldDup"] = _f32(foldDup.astype(np.float32))
    return w


def prep_core_inputs(inp, core):
    b0, b1 = core * B, (core + 1) * B
    x = np.asarray(inp["input_p_q"])[b0:b1, :T, :][:, :, COLS]   # [64,64,17]
    lab = np.asarray(inp["label_p"])[b0:b1, :T]                  # [64,64]
    d = {}
    inpT = np.ones((65, NF * B), np.float32)
    inpT[:64] = x.transpose(1, 2, 0).reshape(64, NF * B)         # [t, (c,b)]
    d["inpT"] = _bf(inpT)
    ct = x.transpose(2, 1, 0).reshape(NF, T * B)                 # [c, (t,b)]
    d["inpCTA"] = _bf(ct[0:9])
    d["inpCTB"] = _bf(ct[9:17])
    d["labelT"] = _f32(lab.T * 2.0)                                    # [t, b]
    return d


DRAM_SPECS = {
    "inpT": ([65, NF * B], BF16), "inpCTA": ([9, T * B], BF16),
    "inpCTB": ([8, T * B], BF16), "labelT": ([T, B], F32),
    "Wi1R": ([65, 64], BF16), "Wi2R": ([65, 64], BF16),
    "We1R": ([256, 64], F32), "We2R": ([256, 64], F32),
    "WhR": ([256, 128], F32), "WxR": ([128, 128], F32), "Wxb": ([1, 128], F32),
    "G1XA": ([9, 512], F32), "b1row": ([1, 512], F32), "G1XB": ([8, 512], F32), "G1H": ([128, 512], F32),
    "G2XA": ([65, 512], F32), "b2row": ([1, 512], F32), "G2XB": ([64, 512], F32), "G2H": ([128, 512], F32),
    "GdX": ([128, 512], F32), "GdH": ([128, 512], F32), "bdrow": ([1, 512], F32),
    "vdup1": ([128, T], BF16), "vdup2": ([128, T], BF16), "vdup3": ([128, H], BF16),
    "regw": ([128, 1], F32), "I64dup": ([128, 64], F32),
    "foldLbf": ([128, 64], BF16), "foldDup": ([128, 128], F32),
}


def build_nc(num_devices=N_CORES, skip_score=False, skip_tail=False, only_stages=(1, 2, 3), split=0.42):
    nc = bacc.Bacc("TRN2", target_bir_lowering=False, debug=False,
                   num_devices=num_devices)
    dr = {}
    for name, (shape, dt) in DRAM_SPECS.items():
        dr[name] = nc.dram_tensor(name, shape, dt, kind="ExternalInput").ap()
    out_d = nc.dram_tensor("out", [B, 18], F32, kind="ExternalOutput").ap()

    with tile.TileContext(nc) as tc:
        # ---------- persistent SBUF ----------
        wpool = tc.alloc_tile_pool(name="wpool", bufs=1)
        sb = {}
        for name, (shape, dt) in DRAM_SPECS.items():
            if shape[0] > 128:
                assert shape[0] == 256
                for half, suf in ((0, "a"), (1, "b")):
                    key = name + suf
                    sb[key] = wpool.tile([128, shape[1]], dt, name=f"sb_{key}")
                    nc.sync.dma_start(sb[key][:], dr[name][128 * half:128 * (half + 1), :])
            else:
                sb[name] = wpool.tile(shape, dt, name=f"sb_{name}")
                nc.sync.dma_start(sb[name][:], dr[name][:])

        X1 = wpool.tile([128, 9, T], BF16, name="X1")
        X2 = wpool.tile([128, 65, T], BF16, name="X2")
        WxF3 = wpool.tile([128, 32, H], BF16, name="WxF3")
        finB = wpool.tile([128, H, 32], BF16, name="finB")
        finT = wpool.tile([128, T, B], F32, name="finT")
        midA = wpool.tile([65, T, B], BF16, name="midA")
        midB = wpool.tile([64, T, B], BF16, name="midB")
        mid2T = wpool.tile([65, B, C2], BF16, name="mid2T")
        zeros128 = wpool.tile([128, 128], F32, name="zeros128")
        zeros64 = zeros128[:, 0:64]
        ones1 = wpool.tile([1, 64], F32, name="ones1")
        outsb = wpool.tile([B, 18], F32, name="outsb")

        nc.vector.memset(zeros128[:], 0.0)
        nc.vector.memset(ones1[:], 1.0)
        nc.vector.memset(mid2T[64:65, :, :], 1.0)
        nc.vector.memset(X2[64:128, 64, :], PAD_NEG)
        nc.vector.memset(X1[64:128, 8, :], PAD_NEG)
        # label -> mid2T[t, b, 128] and midB[63, t, b]
        nc.gpsimd.dma_start(mid2T[0:64, :, 128:129], dr["labelT"][:])
        nc.gpsimd.dma_start(midB[63:64, :, :], dr["labelT"][:])

        if only_stages != (1, 2, 3):
            # profiling variants: init tiles a skipped stage would have written
            nc.vector.memset(finT[:], 0.1)
            nc.vector.memset(finB[:], 0.1)
            nc.vector.memset(midA[:], 0.1)
            nc.vector.memset(midB[:], 0.1)
            nc.vector.memset(mid2T[:], 0.1)
            nc.vector.memset(X2[:], 0.1)
            nc.vector.memset(X1[:], 0.1)
            nc.vector.memset(WxF3[:], 0.1)
            nc.vector.memset(outsb[:], 0.0)

        # ---------- X1 build ----------
        with tc.tile_pool(name="xb1", space="PSUM", bufs=1) as xb:
            x1ps = xb.tile([128, 9, T], F32, name="x1ps")
            for c in range(NF):
                g, ch = (0, c) if c < 9 else (1, c - 9)
                rows = slice(g * 64, g * 64 + 64)
                nc.tensor.matmul(x1ps[rows, ch, :],
                                 sb["inpT"][:, c * B:(c + 1) * B],
                                 sb["Wi1R"][:], start=True, stop=True)
            nc.vector.tensor_copy(X1[0:64, :, :], x1ps[0:64, :, :])
            nc.scalar.copy(X1[64:128, 0:8, :], x1ps[64:128, 0:8, :])

        # ================= helpers =================
        def lstm_block(ps_gates, cB_old, cB_new, hB_new, pool):
            # Doubled-state LSTM: states are hS=2h, cS=2c; i/f/o pre-acts are
            # pre-halved in the weights so tanh(x) here equals 2*sigmoid-1.
            ta = pool.tile([64, 512], F32, name="ta", tag="ta", bufs=2)
            nc.scalar.activation(ta[:], ps_gates[:], AF.Tanh)
            u = pool.tile([64, 128], F32, name="u", tag="u", bufs=2)
            v2 = pool.tile([64, 128], F32, name="v2", tag="v2", bufs=2)
            # u = (tanh(i/2)+1)*tanh(g) = 2*sig(i)*tanh(g)
            nc.vector.scalar_tensor_tensor(u[:], ta[:, 0:128], 1.0,
                                           ta[:, 384:512], op0=OP.add, op1=OP.mult)
            # v = (tanh(f/2)+1)*cS = 4*sig(f)*c
            nc.vector.scalar_tensor_tensor(v2[:], ta[:, 128:256], 1.0,
                                           cB_old[:], op0=OP.add, op1=OP.mult)
            # cS_new = v/2 + u = 2*c_new
            nc.vector.scalar_tensor_tensor(cB_new[:], v2[:], 0.5,
                                           u[:], op0=OP.mult, op1=OP.add)
            tcel = pool.tile([64, 128], F32, name="tcel", tag="tcel", bufs=2)
            nc.scalar.activation(tcel[:], cB_new[:], AF.Tanh, scale=0.5)
            # hS_new = (tanh(o/2)+1)*tanh(c) = 2*h_new
            nc.vector.scalar_tensor_tensor(hB_new[:], ta[:, 256:384], 1.0,
                                           tcel[:], op0=OP.add, op1=OP.mult)


        def softmax_nomax(score, pool, ppool, nch, ptag="tps"):
            # score pad slots (if any) must already be ~-30 so exp ~ 0;
            # accum_out fuses the per-partition sum into the exp pass.
            ex = pool.tile([128, nch], F32, name="ex", tag="sm_ex", bufs=2)
            zs = pool.tile([128, 1], F32, name="zs", tag="sm_zs", bufs=2)
            nc.scalar.activation(ex[:], score[:], AF.Exp, accum_out=zs[:])
            zps = ppool.tile([128, 1], F32, name="zps", tag=ptag,
                             bufs=4 if ptag == "tps" else 3)
            nc.tensor.matmul(zps[:], sb["foldDup"][:], zs[:], start=True, stop=True)
            zr = pool.tile([128, 1], F32, name="zr", tag="sm_zr", bufs=2)
            nc.vector.reciprocal(zr[:], zps[:])
            a = pool.tile([128, nch], F32, name="a", tag="sm_a", bufs=2)
            nc.vector.tensor_scalar_mul(a[:], ex[:], zr[:])
            return a

        def tree_to(dst, src, pool, tag, nch, ntau):
            """sum src [128, nch, ntau] over tau into dst [128, nch] slice."""
            nb = 1
            cur, n, lvl = src, ntau, 0
            while n > 2:
                n //= 2
                nxt = pool.tile([128, nch, n], BF16, name=f"{tag}_{lvl}",
                                tag=f"{tag}_{lvl}", bufs=nb)
                nc.vector.tensor_add(nxt[:], cur[:, :, 0:n], cur[:, :, n:2 * n])
                cur, lvl = nxt, lvl + 1
            nc.vector.tensor_add(dst.unsqueeze(-1), cur[:, :, 0:1], cur[:, :, 1:2])

        def score_chunked(Xs, esb, vdup, nch, ntau, sp, tag, pad_neg=False,
                          nchunks=2):
            """returns score [128, nch] bf16; chunks over ch for engine overlap."""
            score = sp.tile([128, nch], BF16, name="score", tag=f"{tag}_score",
                            bufs=2)
            if nchunks == 1:
                bounds = ((0, nch),)
            elif isinstance(nchunks, float):
                cut = max(1, min(nch - 1, int(round(nch * nchunks))))
                bounds = ((0, cut), (cut, nch))
            elif nchunks == 2:
                half = (nch + 1) // 2
                bounds = ((0, half), (half, nch))
            else:
                q = max(1, nch // nchunks)
                cuts = list(range(0, nch, q))
                bounds = tuple((lo, min(lo + q, nch)) for lo in cuts)
            for lo, hi in bounds:
                w = hi - lo
                nb = 1
                scA = sp.tile([128, w, ntau], BF16, name="scA",
                              tag=f"{tag}_scA{lo}", bufs=nb)
                nc.vector.tensor_add(scA[:], Xs[:, lo:hi, :],
                                     esb[:].unsqueeze(1).broadcast_to([128, w, ntau]))
                scT = sp.tile([128, w, ntau], BF16, name="scT",
                              tag=f"{tag}_scT{lo}", bufs=nb)
                nc.scalar.activation(scT[:], scA[:], AF.Tanh)
                scM = sp.tile([128, w, ntau], BF16, name="scM",
                              tag=f"{tag}_scM{lo}", bufs=nb)
                nc.vector.tensor_mul(scM[:], scT[:],
                                     vdup[:].unsqueeze(1).broadcast_to([128, w, ntau]))
                tree_to(score[:, lo:hi], scM, sp, f"{tag}_tr{lo}", w, ntau)
            if pad_neg:
                # kill the (g=1, ch=nch-1) pad slot before exp
                nc.vector.memset(score[64:128, nch - 1:nch], -30.0)
            return score

        # ================= encoder step =================
        def enc_step(t, stage, sp, pp, st):
            if stage == 1:
                Xs, vdup, WeRa, WeRb = X1, sb["vdup1"], sb["We1Ra"], sb["We1Rb"]
                nch = 9
                GH, GXA, GXB = sb["G1H"], sb["G1XA"], sb["G1XB"]
            else:
                Xs, vdup, WeRa, WeRb = X2, sb["vdup2"], sb["We2Ra"], sb["We2Rb"]
                nch = 65
                GH, GXA, GXB = sb["G2H"], sb["G2XA"], sb["G2XB"]
            hT_old, cT_old, cB_old = st["hT"], st["cT"], st["cB"]

            eps = pp.tile([128, T], F32, name="eps", tag="eps", bufs=2)
            for gb in (0, 64):
                o = eps[gb:gb + 64, :]
                nc.tensor.matmul(o, hT_old[:], WeRa[:], start=True, stop=False)
                nc.tensor.matmul(o, cT_old[:], WeRb[:], start=False, stop=True)
            esb = sp.tile([128, T], BF16, name="esb", tag="esb", bufs=2)
            nc.scalar.copy(esb[:], eps[:])

            if skip_score:
                score = sp.tile([128, nch], BF16, name="score", tag="e_score", bufs=2)
                nc.vector.memset(score[:], 0.1)
            else:
                score = score_chunked(Xs, esb, vdup, nch, T, sp, "e", pad_neg=True,
                                      nchunks=split)
            a = softmax_nomax(score, sp, pp, nch)

            aTA = pp.tile([nch if nch > 9 else 9, 64], F32, name="aTA", tag="tps", bufs=4)
            nc.tensor.transpose(aTA[:], a[0:64, 0:(9 if nch == 9 else nch)], sb["I64dup"][0:64, :])
            aTB = pp.tile([(nch - 1) if nch > 9 else 8, 64], F32, name="aTB", tag="tps", bufs=4)
            nc.tensor.transpose(aTB[:], a[64:128, 0:(8 if nch == 9 else nch - 1)], sb["I64dup"][64:128, :])

            if stage == 1:
                xA = sp.tile([9, 64], F32, name="x1A", tag="xA", bufs=2)
                nc.vector.tensor_mul(xA[:],
                                     sb["inpCTA"][:, t * B:(t + 1) * B], aTA[:])
                xB = sp.tile([8, 64], F32, name="x1B", tag="xB", bufs=2)
                nc.vector.tensor_mul(xB[:], sb["inpCTB"][:, t * B:(t + 1) * B], aTB[:])
                brow = sb["b1row"]
            else:
                xA = sp.tile([65, 64], F32, name="x2A", tag="xA", bufs=2)
                nc.vector.tensor_mul(xA[:], midA[:, t, :], aTA[:])
                xB = sp.tile([64, 64], F32, name="x2B", tag="xB", bufs=2)
                nc.vector.tensor_mul(xB[:], midB[:, t, :], aTB[:])
                brow = sb["b2row"]

            gps = pp.tile([64, 512], F32, name="gps", tag="gps", bufs=2)
            nc.tensor.matmul(gps[:], ones1[:], brow[:], start=True, stop=False)
            nc.tensor.matmul(gps[:], hT_old[:], GH[:], start=False, stop=False)
            nc.tensor.matmul(gps[:], xA[:], GXA[:], start=False, stop=False)
            nc.tensor.matmul(gps[:], xB[:], GXB[:], start=False, stop=True)

            cB_new = sp.tile([64, 128], F32, name="cB", tag="cB", bufs=2)
            hB_new = sp.tile([64, 128], F32, name="hB", tag="hB", bufs=2)
            lstm_block(gps, cB_old, cB_new, hB_new, sp)

            hTps = pp.tile([128, 64], F32, name="hTps", tag="tps", bufs=4)
            nc.tensor.transpose(hTps[:], hB_new[:], sb["I64dup"][0:64, :])
            cTps = pp.tile([128, 64], F32, name="cTps", tag="tps", bufs=4)
            nc.tensor.transpose(cTps[:], cB_new[:], sb["I64dup"][0:64, :])
            cT_sb = sp.tile([128, 64], F32, name="cT_sb", tag="cT", bufs=2)
            nc.scalar.copy(cT_sb[:], cTps[:])

            if stage == 1:
                hT_sb = sp.tile([128, 64], F32, name="hT_sb", tag="hT", bufs=2)
                nc.vector.tensor_copy(hT_sb[:], hTps[:])
                nc.vector.tensor_copy(midA[:, t, :], hTps[0:65, :])
                shps = pp.tile([63, 64], F32, name="shps", tag="tps", bufs=4)
                nc.tensor.transpose(shps[:], hB_new[:, 65:128], sb["I64dup"][0:64, :])
                nc.scalar.copy(midB[0:63, t, :], shps[:])
                hbf = sp.tile([64, 128], BF16, name="hbf", tag="hbf", bufs=2)
                nc.scalar.copy(hbf[:], hB_new[:])
                nc.sync.dma_start(mid2T[t:t + 1, :, 0:128], hbf[:])
                st["hT"] = hT_sb
            else:
                nc.vector.tensor_copy(finT[:, t, :], hTps[:])
                g, sl = divmod(t, 32)
                if g == 0:
                    nc.scalar.copy(finB[0:64, :, sl], hB_new[:].unsqueeze(-1))
                else:
                    shf = pp.tile([128, 128], F32, name="shf", tag="gps", bufs=2)
                    nc.tensor.matmul(shf[64:128, :], sb["I64dup"][0:64, :],
                                     hB_new[:], start=True, stop=True)
                    nc.scalar.copy(finB[64:128, :, sl], shf[64:128, :].unsqueeze(-1))
                st["hT"] = finT[:, t, :]
            st["cT"], st["cB"] = cT_sb, cB_new

        # ---------- stage 1 ----------
        with tc.tile_pool(name="s1sp", bufs=2) as sp, \
             tc.tile_pool(name="s1pp", space="PSUM", bufs=2) as pp:
            st = {"hT": zeros64, "cT": zeros64, "cB": zeros128[0:64, :]}
            for t in range(T if 1 in only_stages else 0):
                enc_step(t, 1, sp, pp, st)

        # ---------- X2 build ----------
        with tc.tile_pool(name="xb2", space="PSUM", bufs=2) as xb2:
            for r in range(4):
                x2ps = xb2.tile([128, 16, T], F32, name="x2ps", tag="x2ps", bufs=2)
                for k in range(16):
                    ch = r * 16 + k
                    nc.tensor.matmul(x2ps[0:64, k, :], mid2T[:, :, ch],
                                     sb["Wi2R"][:], start=True, stop=True)
                    nc.tensor.matmul(x2ps[64:128, k, :], mid2T[:, :, 65 + ch],
                                     sb["Wi2R"][:], start=True, stop=True)
                nc.vector.tensor_copy(X2[:, r * 16:(r + 1) * 16, :], x2ps[:])
            x2ps2 = xb2.tile([64, T], F32, name="x2ps2", tag="x2ps2", bufs=1)
            nc.tensor.matmul(x2ps2[:], mid2T[:, :, 64], sb["Wi2R"][:],
                             start=True, stop=True)
            nc.vector.tensor_copy(X2[0:64, 64, :], x2ps2[:])

        # ---------- stage 2 ----------
        with tc.tile_pool(name="s2sp", bufs=2) as sp, \
             tc.tile_pool(name="s2pp", space="PSUM", bufs=2) as pp:
            st = {"hT": zeros64, "cT": zeros64, "cB": zeros128[0:64, :]}
            for t in range(T if 2 in only_stages else 0):
                enc_step(t, 2, sp, pp, st)

        # ---------- WxF build ----------
        with tc.tile_pool(name="wxb", space="PSUM", bufs=2) as wb:
            for r in range(16):
                g0, sl0 = divmod(r * 4, 32)
                rows = slice(g0 * 64, g0 * 64 + 64)
                wxps = wb.tile([128, 4, H], F32, name="wxps", tag="wxps", bufs=2)
                for j in range(4):
                    nc.tensor.matmul(wxps[rows, j, :], finT[:, r * 4 + j, :],
                                     sb["WxR"][:], start=True, stop=True)
                if r % 2 == 0:
                    nc.vector.tensor_copy(WxF3[rows, sl0:sl0 + 4, :], wxps[rows, :, :])
                else:
                    nc.scalar.copy(WxF3[rows, sl0:sl0 + 4, :], wxps[rows, :, :])

        # ---------- stage 3 ----------
        with tc.tile_pool(name="s3sp", bufs=2) as sp, \
             tc.tile_pool(name="s3pp", space="PSUM", bufs=2) as pp:
            outps = pp.tile([64, 18], F32, name="outps", bufs=1) if 3 in only_stages else None
            hT_old, cT_old = zeros64, zeros64
            cB_old = zeros128[0:64, :]
            for t in range(TD if 3 in only_stages else 0):
                eps = pp.tile([128, H], F32, name="e3ps", tag="eps3", bufs=2)
                for gb in (0, 64):
                    o = eps[gb:gb + 64, :]
                    nc.tensor.matmul(o, ones1[:], sb["Wxb"][:], start=True, stop=False)
                    nc.tensor.matmul(o, hT_old[:], sb["WhRa"][:],
                                     start=False, stop=False)
                    nc.tensor.matmul(o, cT_old[:], sb["WhRb"][:],
                                     start=False, stop=True)
                esb = sp.tile([128, H], BF16, name="e3sb", tag="esb3", bufs=2)
                nc.scalar.copy(esb[:], eps[:])

                if skip_score:
                    score = sp.tile([128, 32], BF16, name="score", tag="d_score", bufs=2)
                    nc.vector.memset(score[:], 0.1)
                else:
                    score = score_chunked(WxF3, esb, sb["vdup3"], 32, H, sp, "d", nchunks=split)
                a = softmax_nomax(score, sp, pp, 32, ptag="tps3")
                abf = sp.tile([128, 32], BF16, name="abf", tag="abf", bufs=2)
                nc.vector.tensor_copy(abf[:], a[:])

                uu = sp.tile([128, H], BF16, name="uu", tag="uu", bufs=2)
                for lo, hi in ((0, 64), (64, H)):
                    ym = sp.tile([128, hi - lo, 32], BF16, name="ym",
                                 tag=f"ym{lo}", bufs=1)
                    nc.vector.tensor_mul(ym[:], finB[:, lo:hi, :],
                                         abf[:].unsqueeze(1).broadcast_to([128, hi - lo, 32]))
                    tree_to(uu[:, lo:hi], ym, sp, f"ctr{lo}", hi - lo, 32)
                dinps = pp.tile([64, H], F32, name="dinps", tag="tps3", bufs=3)
                nc.tensor.matmul(dinps[:], sb["foldLbf"][:], uu[:],
                                 start=True, stop=True)
                dinsb = sp.tile([64, H], F32, name="dinsb", tag="dinsb", bufs=2)
                nc.vector.tensor_copy(dinsb[:], dinps[:])
                dTps = pp.tile([128, 64], F32, name="dTps", tag="tps3", bufs=3)
                nc.tensor.transpose(dTps[:], dinsb[:], sb["I64dup"][0:64, :])
                dinT = sp.tile([128, 64], F32, name="dinT", tag="dinT", bufs=2)
                nc.vector.tensor_copy(dinT[:], dTps[:])

                gps = pp.tile([64, 512], F32, name="g3ps", tag="g3ps", bufs=2)
                nc.tensor.matmul(gps[:], ones1[:], sb["bdrow"][:], start=True, stop=False)
                nc.tensor.matmul(gps[:], hT_old[:], sb["GdH"][:], start=False, stop=False)
                nc.tensor.matmul(gps[:], dinT[:], sb["GdX"][:], start=False, stop=True)

                cB_new = sp.tile([64, 128], F32, name="c3B", tag="c3B", bufs=2)
                hB_new = sp.tile([64, 128], F32, name="h3B", tag="h3B", bufs=2)
                lstm_block(gps, cB_old, cB_new, hB_new, sp)
                cB_old = cB_new

                hTps = pp.tile([128, 64], F32, name="h3Tps", tag="tps3", bufs=3)
                nc.tensor.transpose(hTps[:], hB_new[:], sb["I64dup"][0:64, :])
                cTps = pp.tile([128, 64], F32, name="c3Tps", tag="tps3", bufs=3)
                nc.tensor.transpose(cTps[:], cB_new[:], sb["I64dup"][0:64, :])
                hT_sb = sp.tile([128, 64], F32, name="h3T", tag="h3T", bufs=2)
                nc.vector.tensor_copy(hT_sb[:], hTps[:])
                cT_sb = sp.tile([128, 64], F32, name="c3T", tag="c3T", bufs=2)
                nc.vector.tensor_copy(cT_sb[:], cTps[:])
                hT_old, cT_old = hT_sb, cT_sb

                if t >= TD - 18:
                    j = t - (TD - 18)
                    nc.tensor.matmul(outps[:, j:j + 1], hT_sb[:], sb["regw"][:],
                                     start=True, stop=True)

            if 3 in only_stages:
                nc.vector.tensor_copy(outsb[:], outps[:])
            nc.sync.dma_start(out_d[:], outsb[:])

        wpool.release()

    nc.compile()
    return nc


_NC_CACHE = {}


def kernel(**inputs):
    if "nc" not in _NC_CACHE:
        _NC_CACHE["nc"] = build_nc()
    nc = _NC_CACHE["nc"]
    w = prep_weights({k: np.asarray(v) for k, v in inputs.items()})
    in_maps = []
    for core in range(N_CORES):
        m = dict(w)
        m.update(prep_core_inputs(inputs, core))
        in_maps.append(m)
    res = run_bass_kernel_spmd(nc, in_maps, list(range(N_CORES)))
    out = np.concatenate([res.results[c]["out"] for c in range(N_CORES)], axis=0)
    out = out + np.asarray(inputs["reg_b"])[0]
    return out.astype(np.float32)
